# revision 1
# baseline (speedup 1.0000x reference)
"""DecoderRNN (show-attend-tell) on 8 trn2 NeuronCores.

Strategy (per sharding_hint): data-parallel over batch across the 8
cores — B=32 -> 4 per core; all weights replicated. The recurrence
(attention + LSTMCell) runs as a lax.scan on each core over its batch
shard; the vocab projection is applied to all 20 saved hidden states
at once per core (one [4*20, 512] @ [512, 30522] matmul) since logits
never feed back into the recurrence.

kernel(**inputs) takes FULL unsharded inputs, returns FULL outputs
(preds [32,20,30522] f32, alphas [32,20,49] f32) — matching
reference.reference().
"""

import numpy as np

# Hardcoded problem shapes (self-contained; must not read spec.json)
V = 30522
E = 512
ATT = 512
ENC = 2048
H = 512
B = 32
N = 49
T = 21
M = 8  # cores


def _decode_shard(embeds, features, W_w, W_b, U_w, U_b, A_w, A_b,
                  init_h_w, init_h_b, init_c_w, init_c_b,
                  w_ih, w_hh, b_ih, b_hh, fcn_w, fcn_b):
    """Per-core: embeds [b,T,E], features [b,N,ENC] -> preds [b,T-1,V], alphas [b,T-1,N]."""
    import jax
    import jax.numpy as jnp

    mean_f = features.mean(axis=1)                       # [b, ENC]
    h0 = mean_f @ init_h_w.T + init_h_b                  # [b, H]
    c0 = mean_f @ init_c_w.T + init_c_b
    u_hs = jnp.einsum('bne,ae->bna', features, U_w) + U_b  # [b, N, ATT]

    def step(carry, x_t):
        h, c = carry
        w_ah = h @ W_w.T + W_b                                        # [b, ATT]
        scores = jnp.tanh(u_hs + w_ah[:, None, :]) @ A_w[0] + A_b[0]  # [b, N]
        alpha = jax.nn.softmax(scores, axis=1)
        context = jnp.einsum('bn,bne->be', alpha, features)           # [b, ENC]
        lstm_in = jnp.concatenate([x_t, context], axis=1)             # [b, E+ENC]
        gates = lstm_in @ w_ih.T + b_ih + h @ w_hh.T + b_hh           # [b, 4H]
        i, f, g, o = jnp.split(gates, 4, axis=1)
        c2 = jax.nn.sigmoid(f) * c + jax.nn.sigmoid(i) * jnp.tanh(g)
        h2 = jax.nn.sigmoid(o) * jnp.tanh(c2)
        return (h2, c2), (h2, alpha)

    xs = jnp.swapaxes(embeds[:, :-1], 0, 1)              # [T-1, b, E]
    _, (hs, alphas) = jax.lax.scan(step, (h0, c0), xs)
    # hs: [T-1, b, H] -> one batched vocab projection (decoupled from scan)
    b_sz = hs.shape[1]
    hs_flat = jnp.swapaxes(hs, 0, 1).reshape(b_sz * (T - 1), H)
    preds = (hs_flat @ fcn_w.T + fcn_b).reshape(b_sz, T - 1, V)
    return preds, jnp.swapaxes(alphas, 0, 1)


_CACHE = {}


def _get_pmapped():
    import jax
    if 'fn' not in _CACHE:
        # batch-sharded args: embeds, features (axis 0); weights replicated
        _CACHE['fn'] = jax.pmap(
            _decode_shard,
            in_axes=(0, 0) + (None,) * 16,
            devices=jax.devices()[:M],
        )
    return _CACHE['fn']


def kernel(features, captions, emb, W_w, W_b, U_w, U_b, A_w, A_b,
           init_h_w, init_h_b, init_c_w, init_c_b,
           w_ih, w_hh, b_ih, b_hh, fcn_w, fcn_b):
    features = np.asarray(features, np.float32)
    captions = np.asarray(captions, np.int32)
    emb = np.asarray(emb, np.float32)
    # Embedding lookup is pure indexing — do it host-side so the 62 MB
    # table never ships to the devices (only 32*21 rows are used).
    embeds = emb[captions]                                # [B, T, E]

    ws = [np.asarray(a, np.float32) for a in
          (W_w, W_b, U_w, U_b, A_w, A_b, init_h_w, init_h_b,
           init_c_w, init_c_b, w_ih, w_hh, b_ih, b_hh, fcn_w, fcn_b)]

    bs = B // M
    emb_sh = embeds.reshape(M, bs, T, E)
    feat_sh = features.reshape(M, bs, N, ENC)

    try:
        fn = _get_pmapped()
        preds, alphas = fn(emb_sh, feat_sh, *ws)
        preds = np.asarray(preds).reshape(B, T - 1, V)
        alphas = np.asarray(alphas).reshape(B, T - 1, N)
    except Exception:
        # Fallback: single-device (or CPU) jit, still full correctness.
        import jax
        fn1 = jax.jit(_decode_shard)
        preds, alphas = fn1(embeds, features, *ws)
        preds = np.asarray(preds)
        alphas = np.asarray(alphas)
    return preds.astype(np.float32), alphas.astype(np.float32)
